# revision 8
# baseline (speedup 1.0000x reference)
"""AdaptiveStdPooling2d on 8 TRN2 NeuronCores.

Input  x: [16, 128, 512, 128] f32.  Output: [16, 128, 8, 16] f32.
out[b,c,i,j] = sum_{kw=0..7} std_h(x[b, c, 64*i:64*i+64, 8*j+kw])
with biased variance over the 64-row bin plus EPS=1e-14 inside sqrt.

Sharding: pure data parallel over batch B=16 -> 2 per core, no collectives.

The kernel computes in bf16 (as the previous fp32-HBM version already did
via cast-during-DMA), so the host pre-casts x to bf16 and the device reads
16 of the 32 MiB/core it used to — DMA floor ~94 us at ~358 GB/s/core.

Per core, the 8 slabs [128c, 128h, 128w] ("units", u = b*4 + hc) split:
  - 6 "PE units" (host-transposed to [128h, C, W]): square on ACT or DVE,
    then per-channel ldweights/matmul pairs against a [128,2] 0/1 bin
    selector — FWL weight loads make each pair ~27 ns, so the tensor
    engine does both segmented reductions at ~14 us/unit.  var/std from
    PSUM on DVE/ACT, kw-sum via a second tiny matmul against a [128,16]
    selector (which also lands the result back c-on-partitions).
  - 2 "fold units" (natural layout, read directly from the cast x):
    ACT square, then DVE log-fold (fresh-tile adds, bf16 deep, fp32 tail)
    into per-bin sums; a single batched var/sqrt/kw pass at the end.
Fold units sit early in program order so their batched tail overlaps the
PE units' stream; the last unit is a PE unit with a DVE square (fast tail).
"""

import contextlib

import numpy as np

B, C, H, W = 16, 128, 512, 128
N_CORES = 8
B_LOC = B // N_CORES          # 2 batches per core
H_OUT, W_OUT = 8, 16
KH, KW = H // H_OUT, W // W_OUT   # 64, 8
EPS = 1e-14

PE_UNITS = (1, 2, 3, 5, 6, 7)
SQ_DVE_UNITS = (5, 6, 7)

_CACHE = {}


def make_inputs(x16_loc, pe_units=PE_UNITS):
    """x16_loc [B_LOC, C, H, W] bf16 -> {"xn": ..., "xp": [NP,128,C,W]}."""
    xp_l = []
    for u in range(8):
        b, hc = u // 4, u % 4
        if u in pe_units:
            slab = x16_loc[b, :, hc * 128:(hc + 1) * 128, :]
            xp_l.append(slab.transpose(1, 0, 2))
    xp = (np.ascontiguousarray(np.stack(xp_l)) if xp_l
          else np.zeros((1, 128, C, W), x16_loc.dtype))
    xn = np.ascontiguousarray(x16_loc).reshape(B_LOC, C, 4, 2, KH, W)
    return {"xn": xn, "xp": xp}


def build(loop_reps=1, pe_units=PE_UNITS, sq_dve_units=SQ_DVE_UNITS):
    import concourse.bacc as bacc
    import concourse.mybir as mybir
    from concourse import tile

    f32 = mybir.dt.float32
    bf16 = mybir.dt.bfloat16
    Alu = mybir.AluOpType
    Act = mybir.ActivationFunctionType

    np_ = max(len(pe_units), 1)

    nc = bacc.Bacc(None, target_bir_lowering=False)
    xn_in = nc.declare_dram_parameter("xn", [B_LOC, C, 4, 2, KH, W], bf16,
                                      isOutput=False)
    xp_in = nc.declare_dram_parameter("xp", [np_, 128, C, W], bf16,
                                      isOutput=False)
    out = nc.declare_dram_parameter("out", [B_LOC, C, H_OUT, W_OUT], f32,
                                    isOutput=True)

    with tile.TileContext(nc) as tc:
        with (
            tc.tile_pool(name="xbp", bufs=5) as xbp,
            tc.tile_pool(name="sqp", bufs=3) as sqp,
            tc.tile_pool(name="glp", bufs=2) as glp,
            tc.tile_pool(name="ftp", bufs=2) as ftp,
            tc.tile_pool(name="vbp", bufs=1) as vbp,
            tc.tile_pool(name="ptp", bufs=2) as ptp,
            tc.tile_pool(name="psx", bufs=3, space="PSUM") as psx,
            tc.tile_pool(name="pso", bufs=2, space="PSUM") as pso,
            tc.tile_pool(name="op", bufs=1) as op,
        ):
            fold_bins = []
            for u in range(8):
                if u not in pe_units:
                    fold_bins += [(u // 4, 2 * (u % 4)),
                                  (u // 4, 2 * (u % 4) + 1)]
            oacc = op.tile([C, B_LOC, H_OUT, W_OUT], f32, tag="oacc")
            s1acc = op.tile([C, len(fold_bins), W], f32, tag="s1acc")
            s2acc = op.tile([C, len(fold_bins), W], f32, tag="s2acc")
            eps_t = op.tile([C, 1], f32, tag="eps")
            nc.vector.memset(eps_t[:], float(EPS))
            # 0/1 selector [128h, 2]: col j = 1 iff j == h // 64
            sel2f = op.tile([128, 2], f32, tag="sel2f")
            nc.vector.memset(sel2f[:], 1.0)
            nc.gpsimd.affine_select(
                out=sel2f[:], in_=sel2f[:], pattern=[[-KH, 2]],
                compare_op=Alu.is_ge, fill=0.0, base=0, channel_multiplier=1,
            )
            nc.gpsimd.affine_select(
                out=sel2f[:], in_=sel2f[:], pattern=[[KH, 2]],
                compare_op=Alu.is_ge, fill=0.0, base=KH - 1,
                channel_multiplier=-1,
            )
            sel2b = op.tile([128, 2], bf16, tag="sel2b")
            nc.vector.tensor_copy(sel2b[:], sel2f[:])
            # kw selector [128w, 16]: col j = 1 iff j == w // 8
            kwsel = op.tile([128, W_OUT], f32, tag="kwsel")
            nc.vector.memset(kwsel[:], 1.0)
            nc.gpsimd.affine_select(
                out=kwsel[:], in_=kwsel[:], pattern=[[-KW, W_OUT]],
                compare_op=Alu.is_ge, fill=0.0, base=0, channel_multiplier=1,
            )
            nc.gpsimd.affine_select(
                out=kwsel[:], in_=kwsel[:], pattern=[[KW, W_OUT]],
                compare_op=Alu.is_ge, fill=0.0, base=KW - 1,
                channel_multiplier=-1,
            )

            loop_cm = (tc.For_i(0, loop_reps, 1) if loop_reps > 1
                       else contextlib.nullcontext())
            with loop_cm:
                fi = pi = 0
                for u in range(8):
                    b, hc = u // 4, u % 4
                    if u in pe_units:
                        idx = pi
                        pi += 1
                        ps_x = psx.tile([128, 2 * C], f32, tag="ps_x")
                        ps_q = psx.tile([128, 2 * C], f32, tag="ps_q")
                        for hv in range(2):
                            xb = xbp.tile([128, 64, W], bf16, tag="xb")
                            sqb = sqp.tile([128, 64, W], bf16, tag="sqb")
                            # last half of the last unit: quarter-granular
                            # sub-chunks into the same tile, so the tail is
                            # one quarter's square+matmuls, not a half's
                            nsub = 4 if (u == 7 and hv == 1) else 1
                            rows = 64 // nsub
                            for q in range(nsub):
                                h0 = rows * q
                                nc.sync.dma_start(
                                    out=xb[:, h0:h0 + rows, :],
                                    in_=xp_in[idx, :,
                                              64 * hv + h0:64 * hv + h0 + rows,
                                              :],
                                )
                                if u in sq_dve_units:
                                    nc.vector.tensor_mul(
                                        sqb[:, h0:h0 + rows, :],
                                        xb[:, h0:h0 + rows, :],
                                        xb[:, h0:h0 + rows, :])
                                else:
                                    nc.scalar.activation(
                                        sqb[:, h0:h0 + rows, :],
                                        xb[:, h0:h0 + rows, :], Act.Square)
                                for c0 in range(h0, h0 + rows):
                                    c = 64 * hv + c0
                                    nc.tensor.matmul(
                                        ps_x[:, 2 * c:2 * c + 2],
                                        xb[:, c0, :], sel2b[:],
                                        start=True, stop=True,
                                    )
                                    nc.tensor.matmul(
                                        ps_q[:, 2 * c:2 * c + 2],
                                        sqb[:, c0, :], sel2b[:],
                                        start=True, stop=True,
                                    )
                        mean = ptp.tile([128, 2 * C], f32, tag="pmean")
                        m2 = ptp.tile([128, 2 * C], f32, tag="pm2")
                        var = ptp.tile([128, 2 * C], f32, tag="pvar")
                        pstd = ptp.tile([128, 2, C], f32, tag="pstd")
                        nc.vector.tensor_scalar_mul(mean[:], ps_x[:], 1.0 / KH)
                        nc.vector.tensor_mul(m2[:], mean[:], mean[:])
                        nc.vector.scalar_tensor_tensor(
                            out=var[:], in0=ps_q[:], scalar=1.0 / KH,
                            in1=m2[:], op0=Alu.mult, op1=Alu.subtract,
                        )
                        nc.scalar.activation(
                            pstd[:].transpose([0, 2, 1]),
                            var[:].rearrange("p (c t) -> p c t", t=2),
                            Act.Sqrt, bias=eps_t[:], scale=1.0,
                        )
                        ps_o = pso.tile([128, 2, W_OUT], f32, tag="ps_o")
                        for bin_ in range(2):
                            nc.tensor.matmul(
                                ps_o[:, bin_, :],
                                pstd[:, bin_, :], kwsel[:],
                                start=True, stop=True,
                            )
                        nc.vector.tensor_copy(
                            oacc[:, b, 2 * hc:2 * hc + 2, :], ps_o[:])
                    else:
                        idx = fi
                        fi += 1
                        for t in range(2):
                            ih = 2 * hc + t
                            k = fold_bins.index((b, ih))
                            xb = xbp.tile([C, KH, W], bf16, tag="xb")
                            nc.sync.dma_start(
                                out=xb[:], in_=xn_in[b, :, hc, t, :, :])
                            sqb = sqp.tile([C, KH, W], bf16, tag="sqb")
                            nc.scalar.activation(sqb[:], xb[:], Act.Square)
                            xl1 = glp.tile([C, KH // 2, W], bf16, tag="xl1")
                            nc.vector.tensor_add(
                                xl1[:], xb[:, 0:32, :], xb[:, 32:64, :])
                            # x chain: fresh-tile halving, bf16 deep
                            # (h16/h8 rings shared with the sq chain)
                            x16t = ftp.tile([C, 16, W], bf16, tag="h16")
                            x8t = ftp.tile([C, 8, W], bf16, tag="h8")
                            x4t = ftp.tile([C, 4, W], bf16, tag="x4")
                            x2t = ftp.tile([C, 2, W], bf16, tag="x2")
                            nc.vector.tensor_add(
                                x16t[:], xl1[:, 0:16, :], xl1[:, 16:32, :])
                            nc.vector.tensor_add(
                                x8t[:], x16t[:, 0:8, :], x16t[:, 8:16, :])
                            nc.vector.tensor_add(
                                x4t[:], x8t[:, 0:4, :], x8t[:, 4:8, :])
                            nc.vector.tensor_add(
                                x2t[:], x4t[:, 0:2, :], x4t[:, 2:4, :])
                            nc.vector.tensor_add(
                                s1acc[:, k, :], x2t[:, 0, :], x2t[:, 1, :])
                            # sq chain: fresh tiles, f32 from 4 rows
                            q32t = ftp.tile([C, 32, W], bf16, tag="q32")
                            q16t = ftp.tile([C, 16, W], bf16, tag="h16")
                            q8t = ftp.tile([C, 8, W], bf16, tag="h8")
                            q4t = ftp.tile([C, 4, W], f32, tag="q4")
                            q2t = ftp.tile([C, 2, W], f32, tag="q2")
                            nc.vector.tensor_add(
                                q32t[:], sqb[:, 0:32, :], sqb[:, 32:64, :])
                            nc.vector.tensor_add(
                                q16t[:], q32t[:, 0:16, :], q32t[:, 16:32, :])
                            nc.vector.tensor_add(
                                q8t[:], q16t[:, 0:8, :], q16t[:, 8:16, :])
                            nc.vector.tensor_add(
                                q4t[:], q8t[:, 0:4, :], q8t[:, 4:8, :])
                            nc.vector.tensor_add(
                                q2t[:], q4t[:, 0:2, :], q4t[:, 2:4, :])
                            nc.vector.tensor_add(
                                s2acc[:, k, :], q2t[:, 0, :], q2t[:, 1, :])
                # batched fold VS over all fold bins
                nfb = len(fold_bins)
                fmean = vbp.tile([C, nfb, W], f32, tag="fmean")
                fm2 = vbp.tile([C, nfb, W], f32, tag="fm2")
                fvar = vbp.tile([C, nfb, W], f32, tag="fvar")
                fstd = vbp.tile([C, nfb, W], f32, tag="fstd")
                nc.vector.tensor_scalar_mul(fmean[:], s1acc[:], 1.0 / KH)
                nc.vector.tensor_mul(fm2[:], fmean[:], fmean[:])
                nc.vector.scalar_tensor_tensor(
                    out=fvar[:], in0=s2acc[:], scalar=1.0 / KH,
                    in1=fm2[:], op0=Alu.mult, op1=Alu.subtract,
                )
                nc.scalar.activation(
                    fstd[:].rearrange("p k w -> p (k w)"),
                    fvar[:].rearrange("p k w -> p (k w)"),
                    Act.Sqrt, bias=eps_t[:], scale=1.0,
                )
                for k, (b, ih) in enumerate(fold_bins):
                    nc.vector.reduce_sum(
                        out=oacc[:, b, ih, :],
                        in_=fstd[:, k, :].rearrange("p (g k) -> p g k", k=KW),
                        axis=mybir.AxisListType.X,
                    )
            nc.sync.dma_start(out=out.transpose([1, 0, 2, 3]), in_=oacc[:])
    nc.finalize()
    return nc


def kernel(x):
    import ml_dtypes
    from concourse.bass_utils import run_bass_kernel_spmd

    x = np.ascontiguousarray(np.asarray(x, dtype=np.float32))
    assert x.shape == (B, C, H, W), x.shape
    x16 = x.astype(ml_dtypes.bfloat16)

    if "nc" not in _CACHE:
        _CACHE["nc"] = build()
    nc = _CACHE["nc"]

    in_maps = [
        make_inputs(x16[i * B_LOC:(i + 1) * B_LOC])
        for i in range(N_CORES)
    ]
    last_err = None
    for _ in range(3):
        try:
            res = run_bass_kernel_spmd(nc, in_maps, core_ids=list(range(N_CORES)))
            break
        except Exception as e:  # transient axon/device hiccups
            last_err = e
    else:
        raise last_err
    return np.concatenate([np.asarray(r["out"]) for r in res.results], axis=0)


# revision 10
# speedup vs baseline: 1.0421x; 1.0421x over previous
"""AdaptiveStdPooling2d on 8 TRN2 NeuronCores.

Input  x: [16, 128, 512, 128] f32.  Output: [16, 128, 8, 16] f32.
out[b,c,i,j] = sum_{kw=0..7} std_h(x[b, c, 64*i:64*i+64, 8*j+kw])
with biased variance over the 64-row bin plus EPS=1e-14 inside sqrt.

Sharding: pure data parallel over batch B=16 -> 2 per core, no collectives.

The kernel computes in bf16 (as the previous fp32-HBM version already did
via cast-during-DMA), so the host pre-casts x to bf16 and the device reads
16 of the 32 MiB/core it used to — DMA floor ~94 us at ~358 GB/s/core.

Per core, the 8 slabs [128c, 128h, 128w] ("units", u = b*4 + hc) split:
  - 6 "PE units" (host-transposed to [128h, C, W]): square on ACT or DVE,
    then per-channel ldweights/matmul pairs against a [128,2] 0/1 bin
    selector — FWL weight loads make each pair ~27 ns, so the tensor
    engine does both segmented reductions at ~14 us/unit.  var/std from
    PSUM on DVE/ACT, kw-sum via a second tiny matmul against a [128,16]
    selector (which also lands the result back c-on-partitions).
  - 2 "fold units" (natural layout, read directly from the cast x):
    ACT square, then DVE log-fold (fresh-tile adds, bf16 deep, fp32 tail)
    into per-bin sums; a single batched var/sqrt/kw pass at the end.
Fold units sit early in program order so their batched tail overlaps the
PE units' stream; the last unit is a PE unit with a DVE square (fast tail).
"""

import contextlib

import numpy as np

B, C, H, W = 16, 128, 512, 128
N_CORES = 8
B_LOC = B // N_CORES          # 2 batches per core
H_OUT, W_OUT = 8, 16
KH, KW = H // H_OUT, W // W_OUT   # 64, 8
EPS = 1e-14

PE_UNITS = (1, 2, 3, 5, 6, 7)
SQ_DVE_UNITS = (5, 6, 7)

_CACHE = {}


def make_inputs(x16_loc, pe_units=PE_UNITS):
    """x16_loc [B_LOC, C, H, W] bf16 -> {"xn": ..., "xp": [NP,128,C,W]}."""
    xp_l = []
    for u in range(8):
        b, hc = u // 4, u % 4
        if u in pe_units:
            slab = x16_loc[b, :, hc * 128:(hc + 1) * 128, :]
            xp_l.append(slab.transpose(1, 0, 2))
    xp = (np.ascontiguousarray(np.stack(xp_l)) if xp_l
          else np.zeros((1, 128, C, W), x16_loc.dtype))
    xn = np.ascontiguousarray(x16_loc).reshape(B_LOC, C, 4, 2, KH, W)
    return {"xn": xn, "xp": xp}


def build(loop_reps=1, pe_units=PE_UNITS, sq_dve_units=SQ_DVE_UNITS):
    import concourse.bacc as bacc
    import concourse.mybir as mybir
    from concourse import tile

    f32 = mybir.dt.float32
    bf16 = mybir.dt.bfloat16
    Alu = mybir.AluOpType
    Act = mybir.ActivationFunctionType

    np_ = max(len(pe_units), 1)

    nc = bacc.Bacc(None, target_bir_lowering=False)
    xn_in = nc.declare_dram_parameter("xn", [B_LOC, C, 4, 2, KH, W], bf16,
                                      isOutput=False)
    xp_in = nc.declare_dram_parameter("xp", [np_, 128, C, W], bf16,
                                      isOutput=False)
    out = nc.declare_dram_parameter("out", [B_LOC, C, H_OUT, W_OUT], f32,
                                    isOutput=True)

    with tile.TileContext(nc) as tc:
        with (
            tc.tile_pool(name="xbp", bufs=4) as xbp,
            tc.tile_pool(name="sqp", bufs=3) as sqp,
            tc.tile_pool(name="glp", bufs=2) as glp,
            tc.tile_pool(name="ftp", bufs=2) as ftp,
            tc.tile_pool(name="vbp", bufs=1) as vbp,
            tc.tile_pool(name="ptp", bufs=2) as ptp,
            tc.tile_pool(name="psx", bufs=3, space="PSUM") as psx,
            tc.tile_pool(name="pso", bufs=2, space="PSUM") as pso,
            tc.tile_pool(name="op", bufs=1) as op,
        ):
            fold_bins = []
            for u in range(8):
                if u not in pe_units:
                    fold_bins += [(u // 4, 2 * (u % 4)),
                                  (u // 4, 2 * (u % 4) + 1)]
            oacc = op.tile([C, B_LOC, H_OUT, W_OUT], f32, tag="oacc")
            s1acc = op.tile([C, len(fold_bins), W], f32, tag="s1acc")
            s2acc = op.tile([C, len(fold_bins), W], f32, tag="s2acc")
            eps_t = op.tile([C, 1], f32, tag="eps")
            nc.vector.memset(eps_t[:], float(EPS))
            # 0/1 selector [128h, 2]: col j = 1 iff j == h // 64
            sel2f = op.tile([128, 2], f32, tag="sel2f")
            nc.vector.memset(sel2f[:], 1.0)
            nc.gpsimd.affine_select(
                out=sel2f[:], in_=sel2f[:], pattern=[[-KH, 2]],
                compare_op=Alu.is_ge, fill=0.0, base=0, channel_multiplier=1,
            )
            nc.gpsimd.affine_select(
                out=sel2f[:], in_=sel2f[:], pattern=[[KH, 2]],
                compare_op=Alu.is_ge, fill=0.0, base=KH - 1,
                channel_multiplier=-1,
            )
            sel2b = op.tile([128, 2], bf16, tag="sel2b")
            nc.vector.tensor_copy(sel2b[:], sel2f[:])
            # kw selector [128w, 16]: col j = 1 iff j == w // 8
            kwsel = op.tile([128, W_OUT], f32, tag="kwsel")
            nc.vector.memset(kwsel[:], 1.0)
            nc.gpsimd.affine_select(
                out=kwsel[:], in_=kwsel[:], pattern=[[-KW, W_OUT]],
                compare_op=Alu.is_ge, fill=0.0, base=0, channel_multiplier=1,
            )
            nc.gpsimd.affine_select(
                out=kwsel[:], in_=kwsel[:], pattern=[[KW, W_OUT]],
                compare_op=Alu.is_ge, fill=0.0, base=KW - 1,
                channel_multiplier=-1,
            )

            loop_cm = (tc.For_i(0, loop_reps, 1) if loop_reps > 1
                       else contextlib.nullcontext())
            with loop_cm:
                fi = pi = 0
                for u in range(8):
                    b, hc = u // 4, u % 4
                    if u in pe_units:
                        idx = pi
                        pi += 1
                        ps_x = psx.tile([128, 2 * C], f32, tag="ps_x")
                        ps_q = psx.tile([128, 2 * C], f32, tag="ps_q")
                        for hv in range(2):
                            xb = xbp.tile([128, 64, W], bf16, tag="xb")
                            nc.sync.dma_start(
                                out=xb[:],
                                in_=xp_in[idx, :, 64 * hv:64 * (hv + 1), :],
                            )
                            sqb = sqp.tile([128, 64, W], bf16, tag="sqb")
                            if u in sq_dve_units:
                                nc.vector.tensor_mul(sqb[:], xb[:], xb[:])
                            else:
                                nc.scalar.activation(sqb[:], xb[:], Act.Square)
                            for c0 in range(64):
                                c = 64 * hv + c0
                                nc.tensor.matmul(
                                    ps_x[:, 2 * c:2 * c + 2],
                                    xb[:, c0, :], sel2b[:],
                                    start=True, stop=True,
                                )
                                nc.tensor.matmul(
                                    ps_q[:, 2 * c:2 * c + 2],
                                    sqb[:, c0, :], sel2b[:],
                                    start=True, stop=True,
                                )
                        mean = ptp.tile([128, 2 * C], f32, tag="pmean")
                        m2 = ptp.tile([128, 2 * C], f32, tag="pm2")
                        var = ptp.tile([128, 2 * C], f32, tag="pvar")
                        pstd = ptp.tile([128, 2, C], f32, tag="pstd")
                        nc.vector.tensor_scalar_mul(mean[:], ps_x[:], 1.0 / KH)
                        nc.vector.tensor_mul(m2[:], mean[:], mean[:])
                        nc.vector.scalar_tensor_tensor(
                            out=var[:], in0=ps_q[:], scalar=1.0 / KH,
                            in1=m2[:], op0=Alu.mult, op1=Alu.subtract,
                        )
                        nc.scalar.activation(
                            pstd[:].transpose([0, 2, 1]),
                            var[:].rearrange("p (c t) -> p c t", t=2),
                            Act.Sqrt, bias=eps_t[:], scale=1.0,
                        )
                        ps_o = pso.tile([128, 2, W_OUT], f32, tag="ps_o")
                        for bin_ in range(2):
                            nc.tensor.matmul(
                                ps_o[:, bin_, :],
                                pstd[:, bin_, :], kwsel[:],
                                start=True, stop=True,
                            )
                        nc.vector.tensor_copy(
                            oacc[:, b, 2 * hc:2 * hc + 2, :], ps_o[:])
                    else:
                        idx = fi
                        fi += 1
                        for t in range(2):
                            ih = 2 * hc + t
                            k = fold_bins.index((b, ih))
                            xb = xbp.tile([C, KH, W], bf16, tag="xb")
                            nc.sync.dma_start(
                                out=xb[:], in_=xn_in[b, :, hc, t, :, :])
                            sqb = sqp.tile([C, KH, W], bf16, tag="sqb")
                            nc.scalar.activation(sqb[:], xb[:], Act.Square)
                            xl1 = glp.tile([C, KH // 2, W], bf16, tag="xl1")
                            nc.vector.tensor_add(
                                xl1[:], xb[:, 0:32, :], xb[:, 32:64, :])
                            # x chain: fresh-tile halving, bf16 deep
                            x16t = ftp.tile([C, 16, W], bf16, tag="x16")
                            x8t = ftp.tile([C, 8, W], bf16, tag="x8")
                            x4t = ftp.tile([C, 4, W], bf16, tag="x4")
                            x2t = ftp.tile([C, 2, W], bf16, tag="x2")
                            nc.vector.tensor_add(
                                x16t[:], xl1[:, 0:16, :], xl1[:, 16:32, :])
                            nc.vector.tensor_add(
                                x8t[:], x16t[:, 0:8, :], x16t[:, 8:16, :])
                            nc.vector.tensor_add(
                                x4t[:], x8t[:, 0:4, :], x8t[:, 4:8, :])
                            nc.vector.tensor_add(
                                x2t[:], x4t[:, 0:2, :], x4t[:, 2:4, :])
                            nc.vector.tensor_add(
                                s1acc[:, k, :], x2t[:, 0, :], x2t[:, 1, :])
                            # sq chain: fresh tiles, f32 from 4 rows
                            q32t = ftp.tile([C, 32, W], bf16, tag="q32")
                            q16t = ftp.tile([C, 16, W], bf16, tag="q16")
                            q8t = ftp.tile([C, 8, W], bf16, tag="q8")
                            q4t = ftp.tile([C, 4, W], f32, tag="q4")
                            q2t = ftp.tile([C, 2, W], f32, tag="q2")
                            nc.vector.tensor_add(
                                q32t[:], sqb[:, 0:32, :], sqb[:, 32:64, :])
                            nc.vector.tensor_add(
                                q16t[:], q32t[:, 0:16, :], q32t[:, 16:32, :])
                            nc.vector.tensor_add(
                                q8t[:], q16t[:, 0:8, :], q16t[:, 8:16, :])
                            nc.vector.tensor_add(
                                q4t[:], q8t[:, 0:4, :], q8t[:, 4:8, :])
                            nc.vector.tensor_add(
                                q2t[:], q4t[:, 0:2, :], q4t[:, 2:4, :])
                            nc.vector.tensor_add(
                                s2acc[:, k, :], q2t[:, 0, :], q2t[:, 1, :])
                # batched fold VS over all fold bins
                nfb = len(fold_bins)
                fmean = vbp.tile([C, nfb, W], f32, tag="fmean")
                fm2 = vbp.tile([C, nfb, W], f32, tag="fm2")
                fvar = vbp.tile([C, nfb, W], f32, tag="fvar")
                fstd = vbp.tile([C, nfb, W], f32, tag="fstd")
                nc.vector.tensor_scalar_mul(fmean[:], s1acc[:], 1.0 / KH)
                nc.vector.tensor_mul(fm2[:], fmean[:], fmean[:])
                nc.vector.scalar_tensor_tensor(
                    out=fvar[:], in0=s2acc[:], scalar=1.0 / KH,
                    in1=fm2[:], op0=Alu.mult, op1=Alu.subtract,
                )
                nc.scalar.activation(
                    fstd[:].rearrange("p k w -> p (k w)"),
                    fvar[:].rearrange("p k w -> p (k w)"),
                    Act.Sqrt, bias=eps_t[:], scale=1.0,
                )
                for k, (b, ih) in enumerate(fold_bins):
                    nc.vector.reduce_sum(
                        out=oacc[:, b, ih, :],
                        in_=fstd[:, k, :].rearrange("p (g k) -> p g k", k=KW),
                        axis=mybir.AxisListType.X,
                    )
            nc.sync.dma_start(out=out.transpose([1, 0, 2, 3]), in_=oacc[:])
    nc.finalize()
    return nc


def kernel(x):
    import ml_dtypes
    from concourse.bass_utils import run_bass_kernel_spmd

    x = np.ascontiguousarray(np.asarray(x, dtype=np.float32))
    assert x.shape == (B, C, H, W), x.shape
    x16 = x.astype(ml_dtypes.bfloat16)

    if "nc" not in _CACHE:
        _CACHE["nc"] = build()
    nc = _CACHE["nc"]

    in_maps = [
        make_inputs(x16[i * B_LOC:(i + 1) * B_LOC])
        for i in range(N_CORES)
    ]
    last_err = None
    for _ in range(3):
        try:
            res = run_bass_kernel_spmd(nc, in_maps, core_ids=list(range(N_CORES)))
            break
        except Exception as e:  # transient axon/device hiccups
            last_err = e
    else:
        raise last_err
    return np.concatenate([np.asarray(r["out"]) for r in res.results], axis=0)
